# revision 1
# baseline (speedup 1.0000x reference)
"""DoubleAttention Trainium2 kernel — data-parallel over batch across 8 cores.

Self-contained: takes full inputs, shards n=16 over 8 cores (2 samples/core),
runs a Bass/Tile kernel per core, gathers the full output.

Math per sample (C=512, KC=256, VC=512, H=8 heads, L=4096):
  K = Wk@X, Q = Wq@X, V = Wv@X          (1x1 convs as matmuls)
  key_sm = softmax_L(K + bk) = softmax_L(K)        (bk shift-invariant)
  q_sm   = softmax_head32(Q + bq)
  context_h = V_h @ key_sm_h^T                      (per head, 64x32)
  att = context @ q_sm (block-diag) ; out = x + We@att + be
Folds used by the kernel:
  - context^T computed directly as E_k^T-lhsT matmuls (E=exp(K), layout B)
  - 1/sum_k and per-head block masking folded into context_n
  - M^T = (We @ context_n)^T precomputed once per sample -> output proj
    contracts over 256 (not 512) and absorbs the att matmul
  - bv, be folded into wbias = We@bv + be (host) ; bk dropped (no-op)
All big matmuls in float32r (full PE rate, ~1e-3 rel err).
Sample 1 stage-1 is interleaved with sample 0 phase-A to keep PE dense.
"""

import numpy as np

_CACHE = {}

N_CORES = 8
N, C, Hdim, Wdim = 16, 512, 64, 64
L = Hdim * Wdim            # 4096
KC, VC = 256, 512
NH = 8                     # heads
HV = VC // NH              # 64 head value channels
S_PER_CORE = N // N_CORES  # 2 samples per core
NB = L // 512              # 8 L-banks of 512
NT = L // 128              # 32 L-tiles of 128


def _build_nc():
    import concourse.mybir as mybir
    import concourse.tile as tile
    from concourse import bacc

    F32 = mybir.dt.float32
    F32R = mybir.dt.float32r
    AF = mybir.ActivationFunctionType
    ALU = mybir.AluOpType

    nc = bacc.Bacc("TRN2", target_bir_lowering=False, debug=False)

    # f32r dram views: same bits as f32, lets HWDGE (sync) DMA load without
    # a casting (gpsimd-only) path; PE rounds on consumption.
    xin = nc.dram_tensor("xin", [S_PER_CORE * C, L], F32R, kind="ExternalInput")
    wkT_d = nc.dram_tensor("wkT", [128, 4, KC], F32R, kind="ExternalInput")
    wqT_d = nc.dram_tensor("wqT", [128, 4, KC], F32R, kind="ExternalInput")
    wvT_d = nc.dram_tensor("wvT", [128, 4, VC], F32R, kind="ExternalInput")
    weT_d = nc.dram_tensor("weT", [128, 4, C], F32R, kind="ExternalInput")
    bq_d = nc.dram_tensor("bqv", [128, 2], F32, kind="ExternalInput")
    wb_d = nc.dram_tensor("wbv", [128, 4], F32, kind="ExternalInput")
    g4_d = nc.dram_tensor("g4", [128, 4], F32R, kind="ExternalInput")
    bs_d = nc.dram_tensor("bsum", [128, 128], F32R, kind="ExternalInput")
    idr_d = nc.dram_tensor("identr", [128, 128], F32R, kind="ExternalInput")
    id_d = nc.dram_tensor("ident", [128, 128], F32, kind="ExternalInput")
    ones_d = nc.dram_tensor("ones", [128, 1], F32R, kind="ExternalInput")
    out_d = nc.dram_tensor("out", [S_PER_CORE * C, L], F32, kind="ExternalOutput")

    with tile.TileContext(nc) as tc:
        with tc.tile_pool(name="wpool", bufs=1) as wp, \
             tc.tile_pool(name="work", bufs=1) as sp, \
             tc.tile_pool(name="ppool", bufs=1, space="PSUM") as pp:

            # ---- resident weights/constants ----
            wkT = wp.tile([128, 4, KC], F32R, name="wkT_s")
            wqT = wp.tile([128, 4, KC], F32R, name="wqT_s")
            wvT = wp.tile([128, 4, VC], F32R, name="wvT_s")
            weT = wp.tile([128, 4, C], F32R, name="weT_s")
            g4 = wp.tile([128, 4], F32R, name="g4_s")
            bsum = wp.tile([128, 128], F32R, name="bsum_s")
            idr = wp.tile([128, 128], F32R, name="idr_s")
            ident = wp.tile([128, 128], F32, name="id_s")
            ones = wp.tile([128, 1], F32R, name="ones_s")
            bq = wp.tile([128, 2], F32, name="bq_s")
            wb = wp.tile([128, 4], F32, name="wb_s")

            # first x tile goes out before the weights so DMA queues overlap
            x4_first = sp.tile([128, 4, 512], F32R, name="x4_0_0", tag="x4",
                               bufs=4)
            for c in range(4):
                nc.gpsimd.dma_start(
                    out=x4_first[:, c, :],
                    in_=xin[c * 128:(c + 1) * 128, 0:512])
                nc.sync.dma_start(out=wkT[:, c, :], in_=wkT_d[:, c, :])
                nc.sync.dma_start(out=wvT[:, c, :], in_=wvT_d[:, c, :])
            for dst, src in ((wqT, wqT_d), (weT, weT_d), (g4, g4_d),
                             (bsum, bs_d), (idr, idr_d), (ident, id_d),
                             (ones, ones_d), (bq, bq_d), (wb, wb_d)):
                nc.sync.dma_start(out=dst, in_=src[...])

            st = {}   # per-sample state: ctx_ps, skt_ps, mt

            def stage1_group(s, g, ks=range(4)):
                row0 = s * C
                if g == 0:
                    st[s] = dict(
                        ctx_ps=[pp.tile([128, KC], F32, name=f"ctx{s}_{j}",
                                        tag="ctx", bufs=2) for j in range(2)],
                        skt_ps=pp.tile([1, KC], F32, name=f"skt{s}",
                                       tag="skt", bufs=2))
                if s == 0 and g == 0:
                    x4 = x4_first
                else:
                    x4 = sp.tile([128, 4, 512], F32R, name=f"x4_{s}_{g}",
                                 tag="x4", bufs=4)
                    nc.gpsimd.dma_start(
                        out=x4,
                        in_=xin[row0:row0 + C, g * 512:(g + 1) * 512]
                        .rearrange("(c p) l -> p c l", p=128))
                ctx_ps, skt_ps = st[s]["ctx_ps"], st[s]["skt_ps"]
                st[s]["x4cur"] = x4
                for k in ks:
                    t = 4 * g + k
                    ksl = slice(k * 128, (k + 1) * 128)
                    kt_ps = pp.tile([128, KC], F32, name=f"kt{s}_{t}",
                                    tag="kt", bufs=2)
                    vt_ps = pp.tile([128, VC], F32, name=f"vt{s}_{t}",
                                    tag="vt", bufs=2)
                    # kt/vt interleaved: consecutive matmuls share the same
                    # stationary operand x4[:, c, ksl]
                    for c in range(4):
                        nc.tensor.matmul(kt_ps[:, :], x4[:, c, ksl],
                                         wkT[:, c, :],
                                         start=(c == 0), stop=(c == 3))
                        nc.tensor.matmul(vt_ps[:, :], x4[:, c, ksl],
                                         wvT[:, c, :],
                                         start=(c == 0), stop=(c == 3))
                    ekT = sp.tile([128, KC], F32R, name=f"ek{s}_{t}",
                                  tag="ek", bufs=6)
                    nc.scalar.activation(ekT[:, :], kt_ps[:, :], AF.Exp)
                    vt = sp.tile([128, VC], F32R, name=f"vts{s}_{t}",
                                 tag="vts", bufs=6)
                    nc.vector.tensor_copy(vt[:, :], vt_ps[:, :])
                    for j in range(2):
                        jsl = slice(j * 128, (j + 1) * 128)
                        nc.tensor.matmul(ctx_ps[j][:, :], ekT[:, jsl],
                                         vt[:, j * KC:(j + 1) * KC],
                                         start=(t == 0), stop=(t == NT - 1))
                    nc.tensor.matmul(skt_ps[:, :], ones[:, :], ekT[:, :],
                                     start=(t == 0), stop=(t == NT - 1))

            def stage1_tiles2(s, g):
                # second half of a group, x4 already loaded by first half
                row0 = s * C
                x4 = st[s]["x4cur"]
                ctx_ps, skt_ps = st[s]["ctx_ps"], st[s]["skt_ps"]
                for k in (2, 3):
                    t = 4 * g + k
                    ksl = slice(k * 128, (k + 1) * 128)
                    kt_ps = pp.tile([128, KC], F32, name=f"ktb{s}_{t}",
                                    tag="kt", bufs=2)
                    vt_ps = pp.tile([128, VC], F32, name=f"vtb{s}_{t}",
                                    tag="vt", bufs=2)
                    for c in range(4):
                        nc.tensor.matmul(kt_ps[:, :], x4[:, c, ksl],
                                         wkT[:, c, :],
                                         start=(c == 0), stop=(c == 3))
                        nc.tensor.matmul(vt_ps[:, :], x4[:, c, ksl],
                                         wvT[:, c, :],
                                         start=(c == 0), stop=(c == 3))
                    ekT = sp.tile([128, KC], F32R, name=f"ekb{s}_{t}",
                                  tag="ek", bufs=6)
                    nc.scalar.activation(ekT[:, :], kt_ps[:, :], AF.Exp)
                    vt = sp.tile([128, VC], F32R, name=f"vtsb{s}_{t}",
                                 tag="vts", bufs=6)
                    nc.vector.tensor_copy(vt[:, :], vt_ps[:, :])
                    for j in range(2):
                        jsl = slice(j * 128, (j + 1) * 128)
                        nc.tensor.matmul(ctx_ps[j][:, :], ekT[:, jsl],
                                         vt[:, j * KC:(j + 1) * KC],
                                         start=(t == 0), stop=(t == NT - 1))
                    nc.tensor.matmul(skt_ps[:, :], ones[:, :], ekT[:, :],
                                     start=(t == 0), stop=(t == NT - 1))

            def mid(s):
                ctx_ps, skt_ps = st[s]["ctx_ps"], st[s]["skt_ps"]
                sk_sb = sp.tile([1, KC], F32, name=f"sksb{s}", tag="sksb",
                                bufs=2)
                nc.vector.tensor_copy(sk_sb[:, :], skt_ps[:, :])
                # transpose the (1,256) sum row into (128,2) via two K=1
                # matmuls against a 1x1 identity (lhsT.T @ [1] = column)
                rk_ps = pp.tile([128, 2], F32, name=f"rkps{s}", tag="skt",
                                bufs=2)
                for j in range(2):
                    nc.tensor.matmul(rk_ps[:, j:j + 1],
                                     sk_sb[0:1, j * 128:(j + 1) * 128],
                                     ident[0:1, 0:1],
                                     start=True, stop=True)
                rk = sp.tile([128, 2], F32, name=f"rk{s}", tag="rk", bufs=2)
                nc.vector.reciprocal(rk[:, :], rk_ps[:, :])
                ctn = sp.tile([128, 2, KC], F32, name=f"ctn{s}", tag="ctn",
                              bufs=2)
                nc.vector.memset(ctn[:, :, :], 0.0)
                for h in range(NH):
                    j, gg = h // 4, h % 4
                    pr = slice(32 * gg, 32 * gg + 32)
                    vr = slice(HV * gg, HV * gg + HV)   # local v within chunk
                    nc.vector.tensor_scalar_mul(
                        ctn[pr, j, vr], ctx_ps[j][pr, vr], rk[pr, j:j + 1])
                tr_ps = [pp.tile([128, KC], F32, name=f"tr{s}_{j}", tag="kt",
                                 bufs=2) for j in range(2)]
                for j in range(2):
                    for vcl in range(2):
                        vsl = slice(vcl * 128, (vcl + 1) * 128)
                        nc.tensor.transpose(tr_ps[j][:, vsl], ctn[:, j, vsl],
                                            ident[:, :])
                cn = sp.tile([128, 2, KC], F32R, name=f"cn{s}", tag="cn",
                             bufs=2)
                for j in range(2):
                    jsl = slice(j * 128, (j + 1) * 128)
                    nc.scalar.copy(
                        cn[:, :, jsl],
                        tr_ps[j][:, :].rearrange("p (v q) -> p v q", v=2))
                mt = sp.tile([128, 2, C], F32R, name=f"mt{s}", tag="mt",
                             bufs=2)
                for j in range(2):
                    jsl = slice(j * 128, (j + 1) * 128)
                    mt_ps = pp.tile([128, C], F32, name=f"mtp{s}_{j}",
                                    tag="vt", bufs=2)
                    for vcl in range(2):
                        nc.tensor.matmul(mt_ps[:, :], cn[:, vcl, jsl],
                                         weT[:, 2 * j + vcl, :],
                                         start=(vcl == 0), stop=(vcl == 1))
                    nc.scalar.copy(mt[:, j, :], mt_ps[:, :])
                st[s]["mt"] = mt

            # phase A is software-pipelined: the softmax chain of bank
            # b+1 is emitted before the output stage of bank b, so ACT's
            # FIFO never queues exp() behind oc copies and PE always has
            # matmul work while the DVE/ACT chain completes.
            pend = []

            def _softmaxA(s, b):
                row0 = s * C
                bsl = slice(b * 512, (b + 1) * 512)
                xb = sp.tile([128, 4, 512], F32R, name=f"xb{s}_{b}", tag="xb",
                             bufs=3)
                nc.gpsimd.dma_start(
                    out=xb,
                    in_=xin[row0:row0 + C, bsl]
                    .rearrange("(c p) l -> p c l", p=128))
                eqs = []
                for j in range(2):
                    jsl = slice(j * 128, (j + 1) * 128)
                    q_ps = pp.tile([128, 512], F32, name=f"q{s}_{b}_{j}",
                                   tag="kt", bufs=2)
                    for c in range(4):
                        nc.tensor.matmul(q_ps[:, :], wqT[:, c, jsl],
                                         xb[:, c, :],
                                         start=(c == 0), stop=(c == 3))
                    eq = sp.tile([128, 512], F32R, name=f"eq{s}_{b}_{j}",
                                 tag="eq", bufs=4)
                    nc.scalar.activation(eq[:, :], q_ps[:, :], AF.Exp,
                                         bias=bq[:, j:j + 1])
                    eqs.append(eq)
                qsm = []
                for j in range(2):
                    sq_ps = pp.tile([128, 512], F32, name=f"sq{s}_{b}_{j}",
                                    tag="skt", bufs=2)
                    nc.tensor.matmul(sq_ps[:, :], bsum[:, :], eqs[j][:, :],
                                     start=True, stop=True)
                    rf = sp.tile([128, 512], F32, name=f"rf{s}_{b}_{j}",
                                 tag="rf", bufs=3)
                    nc.vector.reciprocal_approx_fast(rf[:, :], sq_ps[:, :])
                    qs = sp.tile([128, 512], F32R, name=f"qs{s}_{b}_{j}",
                                 tag="qs", bufs=4)
                    nc.vector.tensor_mul(qs[:, :], eqs[j][:, :], rf[:, :])
                    qsm.append(qs)
                return xb, qsm

            def _outputA(s, b, xb, qsm):
                row0 = s * C
                mt = st[s]["mt"]
                bsl = slice(b * 512, (b + 1) * 512)
                for c in range(4):
                    o_ps = pp.tile([128, 512], F32, name=f"o{s}_{b}_{c}",
                                   tag="vt", bufs=2)
                    csl = slice(c * 128, (c + 1) * 128)
                    if c < 2:
                        # residual + wbias on DVE (PE array is the bottleneck)
                        for j in range(2):
                            nc.tensor.matmul(o_ps[:, :], mt[:, j, csl],
                                             qsm[j][:, :],
                                             start=(j == 0), stop=(j == 1))
                        oc = sp.tile([128, 512], F32, name=f"oc{s}_{b}_{c}",
                                     tag="oc", bufs=4)
                        nc.vector.scalar_tensor_tensor(
                            out=oc[:, :], in0=o_ps[:, :],
                            scalar=wb[:, c:c + 1],
                            in1=xb[:, c, :].bitcast(F32),
                            op0=ALU.add, op1=ALU.add)
                    else:
                        # residual folded into PSUM via identity matmul,
                        # wbias via ACT bias-add
                        for j in range(2):
                            nc.tensor.matmul(o_ps[:, :], mt[:, j, csl],
                                             qsm[j][:, :],
                                             start=(j == 0), stop=False)
                        nc.tensor.matmul(o_ps[:, :], idr[:, :], xb[:, c, :],
                                         start=False, stop=True)
                        oc = sp.tile([128, 512], F32, name=f"oc{s}_{b}_{c}",
                                     tag="oc", bufs=4)
                        nc.scalar.add(oc[:, :], o_ps[:, :], wb[:, c:c + 1])
                    nc.sync.dma_start(
                        out=out_d[row0 + c * 128:row0 + (c + 1) * 128, bsl],
                        in_=oc[:, :])

            def phaseA_bank(s, b):
                pend.append((s, b) + _softmaxA(s, b))
                if len(pend) > 1:
                    _outputA(*pend.pop(0))

            def phaseA_flush():
                while pend:
                    _outputA(*pend.pop(0))

            # schedule: keep PE dense by interleaving independent work:
            # s1 stage-1 overlaps mid(0)+phaseA(0); leftover phaseA(0) banks
            # are spread through phaseA(1) so chain stalls are filled.
            for g in range(NB):
                stage1_group(0, g)
            stage1_group(1, 0)
            stage1_group(1, 1)
            mid(0)
            # interleave: s1 stage-1 with the first half of s0 phase-A;
            # defer 4 s0 banks into the tail so it always has two
            # independent chains to alternate between.
            for i in range(2, NB):
                if i >= 4:
                    stage1_group(1, i, ks=(0, 1))
                    pend.append((0, i - 4) + _softmaxA(0, i - 4))
                    stage1_tiles2(1, i)
                    if len(pend) > 1:
                        _outputA(*pend.pop(0))
                else:
                    stage1_group(1, i)
            mid(1)
            phaseA_bank(0, 4)
            for i in range(NB):
                phaseA_bank(1, i)
                if i in (0, 2, 4):
                    phaseA_bank(0, 5 + i // 2)
            phaseA_flush()
    nc.compile()
    return nc


def _host_prep(Wk, bk, Wq, bq, Wv, bv, We, be):
    f = np.float32
    def chunkT(w, nchunk):          # (O, C) -> lhsT layout (128, nchunk, O)
        wt = np.ascontiguousarray(w.T.astype(f))          # (C, O)
        return np.ascontiguousarray(
            wt.reshape(nchunk, 128, w.shape[0]).transpose(1, 0, 2))
    wkT = chunkT(Wk, 4)             # (128, 4, 256)
    wqT = chunkT(Wq, 4)
    wvT = chunkT(Wv, 4)
    weT = chunkT(We, 4)             # We.T chunks over v -> (128, 4, 512)
    bq2 = np.ascontiguousarray(bq.astype(f).reshape(2, 128).T)
    wb = np.ascontiguousarray(
        (We.astype(np.float64) @ bv.astype(np.float64)
         + be.astype(np.float64)).astype(f).reshape(4, 128).T)
    g4 = np.zeros((128, 4), f)
    for p in range(128):
        g4[p, p // 32] = 1.0
    bsum = np.zeros((128, 128), f)
    for p in range(128):
        bsum[p, (p // 32) * 32:(p // 32) * 32 + 32] = 1.0
    ident = np.eye(128, dtype=f)
    ones = np.ones((128, 1), f)
    return dict(wkT=wkT, wqT=wqT, wvT=wvT, weT=weT, bqv=bq2, wbv=wb,
                g4=g4, bsum=bsum, identr=ident, ident=ident,
                ones=ones)


def kernel(x, Wk, bk, Wq, bq, Wv, bv, We, be):
    from concourse.bass_utils import run_bass_kernel_spmd

    assert x.shape == (N, C, Hdim, Wdim), x.shape
    if "nc" not in _CACHE:
        _CACHE["nc"] = _build_nc()
    nc = _CACHE["nc"]

    shared = _host_prep(Wk, bk, Wq, bq, Wv, bv, We, be)
    xf = np.ascontiguousarray(x.astype(np.float32).reshape(N, C, L))
    in_maps = []
    for i in range(N_CORES):
        m = dict(shared)
        m["xin"] = np.ascontiguousarray(
            xf[i * S_PER_CORE:(i + 1) * S_PER_CORE].reshape(S_PER_CORE * C, L))
        in_maps.append(m)

    res = run_bass_kernel_spmd(nc, in_maps, core_ids=list(range(N_CORES)))
    out = np.concatenate(
        [res.results[i]["out"].reshape(S_PER_CORE, C, Hdim, Wdim)
         for i in range(N_CORES)], axis=0)
    return out.astype(np.float32)



# revision 5
# speedup vs baseline: 1.5697x; 1.5697x over previous
"""DoubleAttention Trainium2 kernel — fp8 DoubleRow edition, data-parallel
over batch across 8 cores (2 samples/core).

Math per sample (C=512, KC=256, VC=512, H=8 heads, L=4096):
  K = Wk@X, Q = Wq@X, V = Wv@X          (1x1 convs as matmuls)
  key_sm = softmax_L(K)  (bk per-row shift: no-op)
  q_sm   = softmax_head32(Q + bq)
  ctx_h = V_h @ key_sm_h^T ; att = ctx @ q_sm ; out = x + We@att + wbias

fp8 strategy (rel-err budget 2e-2; attention term is ~1% of |out|):
  - All big matmuls in fp8e4m3 with DoubleRow perf mode (2 K-slices per
    instruction, 2x bf16 rate): K/V proj pair input-channel chunks,
    ctx/skt pair L-tiles, Q pairs channel chunks, output pairs mt halves.
  - Weights pre-scaled x16 on host so fp8 quantization stays in normal
    range; exp() applies scale=1/16; other x16 factors cancel or are
    folded into the final 1/2048 output descale.
  - exp shifted by -1.5 (softmax-invariant) so fp8 eq/ek never overflow.
  - Residual path at fp16: x loaded as fp16 (wbias pre-added on host),
    added either on DVE (scalar_tensor_tensor) or via a 2048*I fp16
    identity matmul into PSUM + ACT copy(scale=1/2048); output fp16.
Schedule: s1 stage-1 interleaved with s0 phase-A (same as f32r baseline).
"""

import numpy as np

_CACHE = {}

N_CORES = 8
N, C, Hdim, Wdim = 16, 512, 64, 64
L = Hdim * Wdim            # 4096
KC, VC = 256, 512
NH = 8                     # heads
HV = VC // NH              # 64 head value channels
S_PER_CORE = N // N_CORES  # 2 samples per core
NB = L // 512              # 8 L-banks of 512
NT = L // 128              # 32 L-tiles of 128
NP = NT // 2               # 16 L-tile pairs

WS = 16.0                  # host weight pre-scale
ESH = -1.5                 # exp shift (softmax-invariant)
QS = 32.0                  # qs = 32*qsm  (fp8 range)
MTS = 0.25                 # mt8 = 0.25 * (256*M) = 64*M
ODS = 1.0 / 2048.0         # output descale: 64*32


def _build_nc():
    import concourse.mybir as mybir
    import concourse.tile as tile
    from concourse import bacc

    F32 = mybir.dt.float32
    F16 = mybir.dt.float16
    F8 = mybir.dt.float8e4
    AF = mybir.ActivationFunctionType
    ALU = mybir.AluOpType
    DR = mybir.MatmulPerfMode.DoubleRow

    nc = bacc.Bacc("TRN2", target_bir_lowering=False, debug=False)

    xin8 = nc.dram_tensor("xin8", [S_PER_CORE * C, L], F8, kind="ExternalInput")
    xin16 = nc.dram_tensor("xin16", [S_PER_CORE * C, L], F16,
                           kind="ExternalInput")
    wk8_d = nc.dram_tensor("wk8", [128, 4, KC], F8, kind="ExternalInput")
    wq8_d = nc.dram_tensor("wq8", [128, 4, KC], F8, kind="ExternalInput")
    wv8_d = nc.dram_tensor("wv8", [128, 4, VC], F8, kind="ExternalInput")
    we8_d = nc.dram_tensor("we8", [128, 4, C], F8, kind="ExternalInput")
    bq_d = nc.dram_tensor("bqv", [128, 2], F32, kind="ExternalInput")
    esh_d = nc.dram_tensor("eshv", [128, 1], F32, kind="ExternalInput")
    bs_d = nc.dram_tensor("bsum8", [128, 128], F8, kind="ExternalInput")
    id_d = nc.dram_tensor("ident", [128, 128], F32, kind="ExternalInput")
    idr_d = nc.dram_tensor("idr16", [128, 128], F16, kind="ExternalInput")
    ones_d = nc.dram_tensor("ones8", [128, 2, 16], F8, kind="ExternalInput")
    out_d = nc.dram_tensor("out", [S_PER_CORE * C, L], F16,
                           kind="ExternalOutput")

    with tile.TileContext(nc) as tc:
        with tc.tile_pool(name="wpool", bufs=1) as wp, \
             tc.tile_pool(name="work", bufs=1) as sp, \
             tc.tile_pool(name="ppool", bufs=1, space="PSUM") as pp:

            # ---- resident weights/constants ----
            wk8 = wp.tile([128, 4, KC], F8, name="wk8_s")
            wq8 = wp.tile([128, 4, KC], F8, name="wq8_s")
            wv8 = wp.tile([128, 4, VC], F8, name="wv8_s")
            we8 = wp.tile([128, 4, C], F8, name="we8_s")
            bsum8 = wp.tile([128, 128], F8, name="bs_s")
            ident = wp.tile([128, 128], F32, name="id_s")
            idr16 = wp.tile([128, 128], F16, name="idr_s")
            ones8 = wp.tile([128, 2, 16], F8, name="ones_s")
            bq2 = wp.tile([128, 2], F32, name="bq_s")
            esh = wp.tile([128, 1], F32, name="esh_s")

            # first x tile goes out before the weights so DMA queues overlap
            x4_first = sp.tile([128, 4, 512], F8, name="x4_0_0", tag="x4",
                               bufs=4)
            for c in range(4):
                nc.gpsimd.dma_start(
                    out=x4_first[:, c, :],
                    in_=xin8[c * 128:(c + 1) * 128, 0:512])
                nc.sync.dma_start(out=wk8[:, c, :], in_=wk8_d[:, c, :])
                nc.sync.dma_start(out=wv8[:, c, :], in_=wv8_d[:, c, :])
            for dst, src in ((wq8, wq8_d), (we8, we8_d), (bsum8, bs_d),
                             (ident, id_d), (idr16, idr_d), (ones8, ones_d),
                             (bq2, bq_d), (esh, esh_d)):
                nc.sync.dma_start(out=dst, in_=src[...])

            st = {}   # per-sample state: ctx_ps, skt_ps, mt8

            def stage1_pair(s, x4, pr, ctx_ps, skt_ps):
                # one L-tile pair: 2 tiles of kt/vt + exp/copy, then
                # DoubleRow ctx/skt accumulation over the pair
                ekT2 = sp.tile([128, 2, KC], F8, name=f"ek{s}_{pr}",
                               tag="ek", bufs=3)
                vt2 = sp.tile([128, 2, VC], F8, name=f"vt2{s}_{pr}",
                              tag="vts", bufs=3)
                for i in (0, 1):
                    k = (2 * pr + i) % 4          # tile within group
                    ksl = slice(k * 128, (k + 1) * 128)
                    kt_ps = pp.tile([128, KC], F32, name=f"kt{s}_{pr}_{i}",
                                    tag="kt", bufs=2)
                    vt_ps = pp.tile([128, VC], F32, name=f"vt{s}_{pr}_{i}",
                                    tag="vt", bufs=2)
                    for cp in (0, 1):
                        cs = slice(2 * cp, 2 * cp + 2)
                        nc.tensor.matmul(kt_ps[:, :], x4[:, cs, ksl],
                                         wk8[:, cs, :], start=(cp == 0),
                                         stop=(cp == 1), perf_mode=DR)
                        nc.tensor.matmul(vt_ps[:, :], x4[:, cs, ksl],
                                         wv8[:, cs, :], start=(cp == 0),
                                         stop=(cp == 1), perf_mode=DR)
                    nc.scalar.activation(ekT2[:, i, :], kt_ps[:, :], AF.Exp,
                                         bias=esh[:, 0:1], scale=1.0 / WS)
                    nc.scalar.copy(vt2[:, i, :], vt_ps[:, :])
                for j in (0, 1):
                    jsl = slice(j * 128, (j + 1) * 128)
                    nc.tensor.matmul(ctx_ps[j][:, :], ekT2[:, :, jsl],
                                     vt2[:, :, j * KC:(j + 1) * KC],
                                     start=(pr == 0), stop=(pr == NP - 1),
                                     perf_mode=DR)
                nc.tensor.matmul(skt_ps[:, :], ones8[:, :, 0:1], ekT2[:, :, :],
                                 start=(pr == 0), stop=(pr == NP - 1),
                                 perf_mode=DR)

            def stage1_group(s, g, pairs=(0, 1)):
                row0 = s * C
                if g == 0 and pairs[0] == 0:
                    st[s] = dict(
                        ctx_ps=[pp.tile([128, KC], F32, name=f"ctx{s}_{j}",
                                        tag="ctx", bufs=2) for j in range(2)],
                        skt_ps=pp.tile([1, KC], F32, name=f"skt{s}",
                                       tag="skt", bufs=2))
                if pairs[0] == 0:
                    if s == 0 and g == 0:
                        x4 = x4_first
                    else:
                        x4 = sp.tile([128, 4, 512], F8, name=f"x4_{s}_{g}",
                                     tag="x4", bufs=4)
                        nc.gpsimd.dma_start(
                            out=x4,
                            in_=xin8[row0:row0 + C, g * 512:(g + 1) * 512]
                            .rearrange("(c p) l -> p c l", p=128))
                    st[s]["x4cur"] = x4
                x4 = st[s]["x4cur"]
                ctx_ps, skt_ps = st[s]["ctx_ps"], st[s]["skt_ps"]
                for p_ in pairs:
                    stage1_pair(s, x4, 2 * g + p_, ctx_ps, skt_ps)

            def mid(s):
                ctx_ps, skt_ps = st[s]["ctx_ps"], st[s]["skt_ps"]
                sk_sb = sp.tile([1, KC], F32, name=f"sksb{s}", tag="sksb",
                                bufs=2)
                nc.vector.tensor_copy(sk_sb[:, :], skt_ps[:, :])
                # transpose the (1,256) sum row into (128,2) via two K=1
                # matmuls against a 1x1 identity
                rk_ps = pp.tile([128, 2], F32, name=f"rkps{s}", tag="skt",
                                bufs=2)
                for j in range(2):
                    nc.tensor.matmul(rk_ps[:, j:j + 1],
                                     sk_sb[0:1, j * 128:(j + 1) * 128],
                                     ident[0:1, 0:1],
                                     start=True, stop=True)
                rk = sp.tile([128, 2], F32, name=f"rk{s}", tag="rk", bufs=2)
                nc.vector.reciprocal(rk[:, :], rk_ps[:, :])
                ctn = sp.tile([128, 2, KC], F32, name=f"ctn{s}", tag="ctn",
                              bufs=2)
                nc.vector.memset(ctn[:, :, :], 0.0)
                for h in range(NH):
                    j, gg = h // 4, h % 4
                    pr = slice(32 * gg, 32 * gg + 32)
                    vr = slice(HV * gg, HV * gg + HV)
                    nc.vector.tensor_scalar_mul(
                        ctn[pr, j, vr], ctx_ps[j][pr, vr], rk[pr, j:j + 1])
                tr_ps = [pp.tile([128, KC], F32, name=f"tr{s}_{j}", tag="kt",
                                 bufs=2) for j in range(2)]
                for j in range(2):
                    for vcl in range(2):
                        vsl = slice(vcl * 128, (vcl + 1) * 128)
                        nc.tensor.transpose(tr_ps[j][:, vsl], ctn[:, j, vsl],
                                            ident[:, :])
                cn = sp.tile([128, 2, KC], F8, name=f"cn{s}", tag="cn",
                             bufs=2)
                for j in range(2):
                    jsl = slice(j * 128, (j + 1) * 128)
                    nc.scalar.copy(
                        cn[:, :, jsl],
                        tr_ps[j][:, :].rearrange("p (v q) -> p v q", v=2))
                mt8 = sp.tile([128, 2, C], F8, name=f"mt{s}", tag="mt",
                              bufs=2)
                for j in range(2):
                    jsl = slice(j * 128, (j + 1) * 128)
                    mt_ps = pp.tile([128, C], F32, name=f"mtp{s}_{j}",
                                    tag="vt", bufs=2)
                    nc.tensor.matmul(mt_ps[:, :], cn[:, :, jsl],
                                     we8[:, 2 * j:2 * j + 2, :],
                                     start=True, stop=True, perf_mode=DR)
                    nc.scalar.activation(mt8[:, j, :], mt_ps[:, :], AF.Copy,
                                         scale=MTS)
                st[s]["mt8"] = mt8

            pend = []

            def _softmaxA(s, b):
                row0 = s * C
                bsl = slice(b * 512, (b + 1) * 512)
                xb8 = sp.tile([128, 4, 512], F8, name=f"xb8{s}_{b}",
                              tag="xb8", bufs=3)
                nc.gpsimd.dma_start(
                    out=xb8,
                    in_=xin8[row0:row0 + C, bsl]
                    .rearrange("(c p) l -> p c l", p=128))
                xb16 = sp.tile([128, 4, 512], F16, name=f"xb{s}_{b}",
                               tag="xb", bufs=3)
                nc.gpsimd.dma_start(
                    out=xb16,
                    in_=xin16[row0:row0 + C, bsl]
                    .rearrange("(c p) l -> p c l", p=128))
                qs2 = sp.tile([128, 2, 512], F8, name=f"qs{s}_{b}",
                              tag="qs", bufs=4)
                for j in range(2):
                    jsl = slice(j * 128, (j + 1) * 128)
                    q_ps = pp.tile([128, 512], F32, name=f"q{s}_{b}_{j}",
                                   tag="kt", bufs=2)
                    for cp in (0, 1):
                        cs = slice(2 * cp, 2 * cp + 2)
                        nc.tensor.matmul(q_ps[:, :], wq8[:, cs, jsl],
                                         xb8[:, cs, :], start=(cp == 0),
                                         stop=(cp == 1), perf_mode=DR)
                    eq = sp.tile([128, 512], F8, name=f"eq{s}_{b}_{j}",
                                 tag="eq", bufs=4)
                    nc.scalar.activation(eq[:, :], q_ps[:, :], AF.Exp,
                                         bias=bq2[:, j:j + 1],
                                         scale=1.0 / WS)
                    sq_ps = pp.tile([128, 512], F32, name=f"sq{s}_{b}_{j}",
                                    tag="skt", bufs=2)
                    nc.tensor.matmul(sq_ps[:, :], bsum8[:, :], eq[:, :],
                                     start=True, stop=True)
                    rf = sp.tile([128, 512], F32, name=f"rf{s}_{b}_{j}",
                                 tag="rf", bufs=3)
                    nc.vector.reciprocal_approx_fast(rf[:, :], sq_ps[:, :])
                    nc.vector.scalar_tensor_tensor(
                        out=qs2[:, j, :], in0=eq[:, :], scalar=QS,
                        in1=rf[:, :], op0=ALU.mult, op1=ALU.mult)
                return xb16, qs2

            def _outputA(s, b, xb16, qs2):
                row0 = s * C
                mt8 = st[s]["mt8"]
                bsl = slice(b * 512, (b + 1) * 512)
                for c in range(4):
                    o_ps = pp.tile([128, 512], F32, name=f"o{s}_{b}_{c}",
                                   tag="vt", bufs=2)
                    csl = slice(c * 128, (c + 1) * 128)
                    if c < 1:
                        # residual on DVE
                        nc.tensor.matmul(o_ps[:, :], mt8[:, :, csl],
                                         qs2[:, :, :], start=True, stop=True,
                                         perf_mode=DR)
                        oc = sp.tile([128, 512], F16, name=f"oc{s}_{b}_{c}",
                                     tag="oc", bufs=4)
                        nc.vector.scalar_tensor_tensor(
                            out=oc[:, :], in0=o_ps[:, :], scalar=ODS,
                            in1=xb16[:, c, :], op0=ALU.mult, op1=ALU.add)
                    else:
                        # residual folded into PSUM via 2048*I fp16 matmul,
                        # descale via ACT copy
                        nc.tensor.matmul(o_ps[:, :], mt8[:, :, csl],
                                         qs2[:, :, :], start=True, stop=False,
                                         perf_mode=DR)
                        nc.tensor.matmul(o_ps[:, :], idr16[:, :],
                                         xb16[:, c, :],
                                         start=False, stop=True)
                        oc = sp.tile([128, 512], F16, name=f"oc{s}_{b}_{c}",
                                     tag="oc", bufs=4)
                        nc.scalar.activation(oc[:, :], o_ps[:, :], AF.Copy,
                                             scale=ODS)
                    nc.sync.dma_start(
                        out=out_d[row0 + c * 128:row0 + (c + 1) * 128, bsl],
                        in_=oc[:, :])

            def phaseA_bank(s, b):
                pend.append((s, b) + _softmaxA(s, b))
                if len(pend) > 1:
                    _outputA(*pend.pop(0))

            def phaseA_flush():
                while pend:
                    _outputA(*pend.pop(0))

            # schedule: identical shape to the f32r baseline
            for g in range(NB):
                stage1_group(0, g)
            stage1_group(1, 0)
            stage1_group(1, 1)
            mid(0)
            for i in range(2, NB):
                if i >= 4:
                    stage1_group(1, i, pairs=(0,))
                    pend.append((0, i - 4) + _softmaxA(0, i - 4))
                    stage1_group(1, i, pairs=(1,))
                    if len(pend) > 1:
                        _outputA(*pend.pop(0))
                else:
                    stage1_group(1, i)
            mid(1)
            phaseA_bank(0, 4)
            for i in range(NB):
                phaseA_bank(1, i)
                if i in (0, 2, 4):
                    phaseA_bank(0, 5 + i // 2)
            phaseA_flush()
    nc.compile()
    return nc


def _host_prep(Wk, bk, Wq, bq, Wv, bv, We, be):
    import ml_dtypes
    f = np.float32
    F8 = ml_dtypes.float8_e4m3

    def chunk8(w):                  # (O, Cin) -> (128, Cin//128, O) fp8, x16
        wt = np.ascontiguousarray(w.T.astype(np.float64) * WS)
        wt = np.clip(wt, -240.0, 240.0)
        nch = wt.shape[0] // 128
        return np.ascontiguousarray(
            wt.reshape(nch, 128, w.shape[0]).transpose(1, 0, 2)).astype(F8)

    wk8 = chunk8(Wk)
    wq8 = chunk8(Wq)
    wv8 = chunk8(Wv)
    we8 = chunk8(We)
    bq2 = np.ascontiguousarray(
        bq.astype(f).reshape(2, 128).T) + np.float32(ESH)
    wb = (We.astype(np.float64) @ bv.astype(np.float64)
          + be.astype(np.float64))
    bsum = np.zeros((128, 128), f)
    for p in range(128):
        bsum[p, (p // 32) * 32:(p // 32) * 32 + 32] = 1.0
    ident = np.eye(128, dtype=f)
    idr16 = (np.eye(128) * 2048.0).astype(np.float16)
    ones8 = np.ones((128, 2, 16), dtype=F8)
    eshv = np.full((128, 1), ESH, dtype=f)
    return dict(wk8=wk8, wq8=wq8, wv8=wv8, we8=we8, bqv=bq2, eshv=eshv,
                bsum8=bsum.astype(F8), ident=ident, idr16=idr16,
                ones8=ones8), wb


def _make_in_maps(x, Wk, bk, Wq, bq, Wv, bv, We, be):
    import ml_dtypes
    F8 = ml_dtypes.float8_e4m3
    shared, wb = _host_prep(Wk, bk, Wq, bq, Wv, bv, We, be)
    xf = np.ascontiguousarray(x.astype(np.float64).reshape(N, C, L))
    x8 = np.clip(xf, -240.0, 240.0).astype(F8)
    x16 = (xf + wb[None, :, None]).astype(np.float16)
    in_maps = []
    for i in range(N_CORES):
        m = dict(shared)
        sl = slice(i * S_PER_CORE, (i + 1) * S_PER_CORE)
        m["xin8"] = np.ascontiguousarray(
            x8[sl].reshape(S_PER_CORE * C, L))
        m["xin16"] = np.ascontiguousarray(
            x16[sl].reshape(S_PER_CORE * C, L))
        in_maps.append(m)
    return in_maps


def kernel(x, Wk, bk, Wq, bq, Wv, bv, We, be):
    from concourse.bass_utils import run_bass_kernel_spmd

    assert x.shape == (N, C, Hdim, Wdim), x.shape
    if "nc" not in _CACHE:
        _CACHE["nc"] = _build_nc()
    nc = _CACHE["nc"]

    in_maps = _make_in_maps(x, Wk, bk, Wq, bq, Wv, bv, We, be)
    res = run_bass_kernel_spmd(nc, in_maps, core_ids=list(range(N_CORES)))
    out = np.concatenate(
        [np.asarray(res.results[i]["out"], dtype=np.float32)
         .reshape(S_PER_CORE, C, Hdim, Wdim)
         for i in range(N_CORES)], axis=0)
    return out.astype(np.float32)


# revision 6
# speedup vs baseline: 1.6086x; 1.0248x over previous
"""DoubleAttention Trainium2 kernel — fp8 DoubleRow edition, data-parallel
over batch across 8 cores (2 samples/core).

Math per sample (C=512, KC=256, VC=512, H=8 heads, L=4096):
  K = Wk@X, Q = Wq@X, V = Wv@X          (1x1 convs as matmuls)
  key_sm = softmax_L(K)  (bk per-row shift: no-op)
  q_sm   = softmax_head32(Q + bq)
  ctx_h = V_h @ key_sm_h^T ; att = ctx @ q_sm ; out = x + We@att + wbias

fp8 strategy (rel-err budget 2e-2; attention term is ~1% of |out|):
  - All big matmuls in fp8e4m3 with DoubleRow perf mode (2 K-slices per
    instruction, 2x bf16 rate): K/V proj pair input-channel chunks,
    ctx/skt pair L-tiles, Q pairs channel chunks, output pairs mt halves.
  - Weights pre-scaled x16 on host so fp8 quantization stays in normal
    range; exp() applies scale=1/16; other x16 factors cancel or are
    folded into the final 1/2048 output descale.
  - exp shifted by -1.5 (softmax-invariant) so fp8 eq/ek never overflow.
  - Residual path at fp16: x loaded as fp16 (wbias pre-added on host),
    added either on DVE (scalar_tensor_tensor) or via a 2048*I fp16
    identity matmul into PSUM + ACT copy(scale=1/2048); output fp16.
Schedule: s1 stage-1 interleaved with s0 phase-A (same as f32r baseline).
"""

import numpy as np

_CACHE = {}

N_CORES = 8
N, C, Hdim, Wdim = 16, 512, 64, 64
L = Hdim * Wdim            # 4096
KC, VC = 256, 512
NH = 8                     # heads
HV = VC // NH              # 64 head value channels
S_PER_CORE = N // N_CORES  # 2 samples per core
NB = L // 512              # 8 L-banks of 512
NT = L // 128              # 32 L-tiles of 128
NP = NT // 2               # 16 L-tile pairs

WS = 16.0                  # host weight pre-scale
ESH = -1.5                 # exp shift (softmax-invariant)
QS = 32.0                  # qs = 32*qsm  (fp8 range)
MTS = 0.25                 # mt8 = 0.25 * (256*M) = 64*M
ODS = 1.0 / 2048.0         # output descale: 64*32


def _build_nc():
    import concourse.mybir as mybir
    import concourse.tile as tile
    from concourse import bacc

    F32 = mybir.dt.float32
    F16 = mybir.dt.float16
    F8 = mybir.dt.float8e4
    AF = mybir.ActivationFunctionType
    ALU = mybir.AluOpType
    DR = mybir.MatmulPerfMode.DoubleRow

    nc = bacc.Bacc("TRN2", target_bir_lowering=False, debug=False)

    xin8 = nc.dram_tensor("xin8", [S_PER_CORE * C, L], F8, kind="ExternalInput")
    xin16 = nc.dram_tensor("xin16", [S_PER_CORE * C, L], F16,
                           kind="ExternalInput")
    wk8_d = nc.dram_tensor("wk8", [128, 4, KC], F8, kind="ExternalInput")
    wq8_d = nc.dram_tensor("wq8", [128, 4, KC], F8, kind="ExternalInput")
    wv8_d = nc.dram_tensor("wv8", [128, 4, VC], F8, kind="ExternalInput")
    we8_d = nc.dram_tensor("we8", [128, 4, C], F8, kind="ExternalInput")
    bq_d = nc.dram_tensor("bqv", [128, 2], F32, kind="ExternalInput")
    esh_d = nc.dram_tensor("eshv", [128, 1], F32, kind="ExternalInput")
    bs_d = nc.dram_tensor("bsum8", [128, 128], F8, kind="ExternalInput")
    id_d = nc.dram_tensor("ident", [128, 128], F32, kind="ExternalInput")
    idr_d = nc.dram_tensor("idr16", [128, 128], F16, kind="ExternalInput")
    ones_d = nc.dram_tensor("ones8", [128, 2, 16], F8, kind="ExternalInput")
    out_d = nc.dram_tensor("out", [S_PER_CORE * C, L], F16,
                           kind="ExternalOutput")

    with tile.TileContext(nc) as tc:
        with tc.tile_pool(name="wpool", bufs=1) as wp, \
             tc.tile_pool(name="work", bufs=1) as sp, \
             tc.tile_pool(name="ppool", bufs=1, space="PSUM") as pp:

            # ---- resident weights/constants ----
            wk8 = wp.tile([128, 4, KC], F8, name="wk8_s")
            wq8 = wp.tile([128, 4, KC], F8, name="wq8_s")
            wv8 = wp.tile([128, 4, VC], F8, name="wv8_s")
            we8 = wp.tile([128, 4, C], F8, name="we8_s")
            bsum8 = wp.tile([128, 128], F8, name="bs_s")
            ident = wp.tile([128, 128], F32, name="id_s")
            idr16 = wp.tile([128, 128], F16, name="idr_s")
            ones8 = wp.tile([128, 2, 16], F8, name="ones_s")
            bq2 = wp.tile([128, 2], F32, name="bq_s")
            esh = wp.tile([128, 1], F32, name="esh_s")

            # first x tile goes out before the weights so DMA queues overlap
            x4_first = sp.tile([128, 4, 512], F8, name="x4_0_0", tag="x4",
                               bufs=4)
            for c in range(4):
                nc.gpsimd.dma_start(
                    out=x4_first[:, c, :],
                    in_=xin8[c * 128:(c + 1) * 128, 0:512])
                nc.sync.dma_start(out=wk8[:, c, :], in_=wk8_d[:, c, :])
                nc.sync.dma_start(out=wv8[:, c, :], in_=wv8_d[:, c, :])
            for dst, src in ((wq8, wq8_d), (we8, we8_d), (bsum8, bs_d),
                             (ident, id_d), (idr16, idr_d), (ones8, ones_d),
                             (bq2, bq_d), (esh, esh_d)):
                nc.sync.dma_start(out=dst, in_=src[...])

            st = {}   # per-sample state: ctx_ps, skt_ps, mt8

            def stage1_pair(s, x4, pr, ctx_ps, skt_ps):
                # one L-tile pair: 2 tiles of kt/vt + exp/copy, then
                # DoubleRow ctx/skt accumulation over the pair
                ekT2 = sp.tile([128, 2, KC], F8, name=f"ek{s}_{pr}",
                               tag="ek", bufs=3)
                vt2 = sp.tile([128, 2, VC], F8, name=f"vt2{s}_{pr}",
                              tag="vts", bufs=3)
                for i in (0, 1):
                    k = (2 * pr + i) % 4          # tile within group
                    ksl = slice(k * 128, (k + 1) * 128)
                    kt_ps = pp.tile([128, KC], F32, name=f"kt{s}_{pr}_{i}",
                                    tag="kt", bufs=2)
                    vt_ps = pp.tile([128, VC], F32, name=f"vt{s}_{pr}_{i}",
                                    tag="vt", bufs=2)
                    for cp in (0, 1):
                        cs = slice(2 * cp, 2 * cp + 2)
                        nc.tensor.matmul(kt_ps[:, :], x4[:, cs, ksl],
                                         wk8[:, cs, :], start=(cp == 0),
                                         stop=(cp == 1), perf_mode=DR)
                        nc.tensor.matmul(vt_ps[:, :], x4[:, cs, ksl],
                                         wv8[:, cs, :], start=(cp == 0),
                                         stop=(cp == 1), perf_mode=DR)
                    nc.scalar.activation(ekT2[:, i, :], kt_ps[:, :], AF.Exp,
                                         bias=esh[:, 0:1], scale=1.0 / WS)
                    if i == 0:
                        nc.vector.tensor_copy(vt2[:, i, :], vt_ps[:, :])
                    else:
                        nc.scalar.copy(vt2[:, i, :], vt_ps[:, :])
                for j in (0, 1):
                    jsl = slice(j * 128, (j + 1) * 128)
                    nc.tensor.matmul(ctx_ps[j][:, :], ekT2[:, :, jsl],
                                     vt2[:, :, j * KC:(j + 1) * KC],
                                     start=(pr == 0), stop=(pr == NP - 1),
                                     perf_mode=DR)
                nc.tensor.matmul(skt_ps[:, :], ones8[:, :, 0:1], ekT2[:, :, :],
                                 start=(pr == 0), stop=(pr == NP - 1),
                                 perf_mode=DR)

            def stage1_group(s, g, pairs=(0, 1)):
                row0 = s * C
                if g == 0 and pairs[0] == 0:
                    st[s] = dict(
                        ctx_ps=[pp.tile([128, KC], F32, name=f"ctx{s}_{j}",
                                        tag="ctx", bufs=2) for j in range(2)],
                        skt_ps=pp.tile([1, KC], F32, name=f"skt{s}",
                                       tag="skt", bufs=2))
                if pairs[0] == 0:
                    if s == 0 and g == 0:
                        x4 = x4_first
                    else:
                        x4 = sp.tile([128, 4, 512], F8, name=f"x4_{s}_{g}",
                                     tag="x4", bufs=4)
                        nc.gpsimd.dma_start(
                            out=x4,
                            in_=xin8[row0:row0 + C, g * 512:(g + 1) * 512]
                            .rearrange("(c p) l -> p c l", p=128))
                    st[s]["x4cur"] = x4
                x4 = st[s]["x4cur"]
                ctx_ps, skt_ps = st[s]["ctx_ps"], st[s]["skt_ps"]
                for p_ in pairs:
                    stage1_pair(s, x4, 2 * g + p_, ctx_ps, skt_ps)

            def mid(s):
                ctx_ps, skt_ps = st[s]["ctx_ps"], st[s]["skt_ps"]
                sk_sb = sp.tile([1, KC], F32, name=f"sksb{s}", tag="sksb",
                                bufs=2)
                nc.vector.tensor_copy(sk_sb[:, :], skt_ps[:, :])
                # transpose the (1,256) sum row into (128,2) via two K=1
                # matmuls against a 1x1 identity
                rk_ps = pp.tile([128, 2], F32, name=f"rkps{s}", tag="skt",
                                bufs=2)
                for j in range(2):
                    nc.tensor.matmul(rk_ps[:, j:j + 1],
                                     sk_sb[0:1, j * 128:(j + 1) * 128],
                                     ident[0:1, 0:1],
                                     start=True, stop=True)
                rk = sp.tile([128, 2], F32, name=f"rk{s}", tag="rk", bufs=2)
                nc.vector.reciprocal(rk[:, :], rk_ps[:, :])
                ctn = sp.tile([128, 2, KC], F32, name=f"ctn{s}", tag="ctn",
                              bufs=2)
                nc.vector.memset(ctn[:, :, :], 0.0)
                for h in range(NH):
                    j, gg = h // 4, h % 4
                    pr = slice(32 * gg, 32 * gg + 32)
                    vr = slice(HV * gg, HV * gg + HV)
                    nc.vector.tensor_scalar_mul(
                        ctn[pr, j, vr], ctx_ps[j][pr, vr], rk[pr, j:j + 1])
                tr_ps = [pp.tile([128, KC], F32, name=f"tr{s}_{j}", tag="kt",
                                 bufs=2) for j in range(2)]
                for j in range(2):
                    for vcl in range(2):
                        vsl = slice(vcl * 128, (vcl + 1) * 128)
                        nc.tensor.transpose(tr_ps[j][:, vsl], ctn[:, j, vsl],
                                            ident[:, :])
                cn = sp.tile([128, 2, KC], F8, name=f"cn{s}", tag="cn",
                             bufs=2)
                for j in range(2):
                    jsl = slice(j * 128, (j + 1) * 128)
                    nc.scalar.copy(
                        cn[:, :, jsl],
                        tr_ps[j][:, :].rearrange("p (v q) -> p v q", v=2))
                mt8 = sp.tile([128, 2, C], F8, name=f"mt{s}", tag="mt",
                              bufs=2)
                for j in range(2):
                    jsl = slice(j * 128, (j + 1) * 128)
                    mt_ps = pp.tile([128, C], F32, name=f"mtp{s}_{j}",
                                    tag="vt", bufs=2)
                    nc.tensor.matmul(mt_ps[:, :], cn[:, :, jsl],
                                     we8[:, 2 * j:2 * j + 2, :],
                                     start=True, stop=True, perf_mode=DR)
                    nc.scalar.activation(mt8[:, j, :], mt_ps[:, :], AF.Copy,
                                         scale=MTS)
                st[s]["mt8"] = mt8

            pend = []

            def _softmaxA(s, b):
                row0 = s * C
                bsl = slice(b * 512, (b + 1) * 512)
                xb8 = sp.tile([128, 4, 512], F8, name=f"xb8{s}_{b}",
                              tag="xb8", bufs=4)
                nc.gpsimd.dma_start(
                    out=xb8,
                    in_=xin8[row0:row0 + C, bsl]
                    .rearrange("(c p) l -> p c l", p=128))
                xb16 = sp.tile([128, 4, 512], F16, name=f"xb{s}_{b}",
                               tag="xb", bufs=4)
                nc.gpsimd.dma_start(
                    out=xb16,
                    in_=xin16[row0:row0 + C, bsl]
                    .rearrange("(c p) l -> p c l", p=128))
                qs2 = sp.tile([128, 2, 512], F8, name=f"qs{s}_{b}",
                              tag="qs", bufs=6)
                for j in range(2):
                    jsl = slice(j * 128, (j + 1) * 128)
                    q_ps = pp.tile([128, 512], F32, name=f"q{s}_{b}_{j}",
                                   tag="kt", bufs=2)
                    for cp in (0, 1):
                        cs = slice(2 * cp, 2 * cp + 2)
                        nc.tensor.matmul(q_ps[:, :], wq8[:, cs, jsl],
                                         xb8[:, cs, :], start=(cp == 0),
                                         stop=(cp == 1), perf_mode=DR)
                    eq = sp.tile([128, 512], F8, name=f"eq{s}_{b}_{j}",
                                 tag="eq", bufs=6)
                    nc.scalar.activation(eq[:, :], q_ps[:, :], AF.Exp,
                                         bias=bq2[:, j:j + 1],
                                         scale=1.0 / WS)
                    sq_ps = pp.tile([128, 512], F32, name=f"sq{s}_{b}_{j}",
                                    tag="skt", bufs=2)
                    nc.tensor.matmul(sq_ps[:, :], bsum8[:, :], eq[:, :],
                                     start=True, stop=True)
                    rf = sp.tile([128, 512], F32, name=f"rf{s}_{b}_{j}",
                                 tag="rf", bufs=4)
                    nc.vector.reciprocal_approx_fast(rf[:, :], sq_ps[:, :])
                    nc.vector.scalar_tensor_tensor(
                        out=qs2[:, j, :], in0=eq[:, :], scalar=QS,
                        in1=rf[:, :], op0=ALU.mult, op1=ALU.mult)
                return xb16, qs2

            def _outputA(s, b, xb16, qs2):
                row0 = s * C
                mt8 = st[s]["mt8"]
                bsl = slice(b * 512, (b + 1) * 512)
                for c in range(4):
                    o_ps = pp.tile([128, 512], F32, name=f"o{s}_{b}_{c}",
                                   tag="vt", bufs=2)
                    csl = slice(c * 128, (c + 1) * 128)
                    if c < 2:
                        # residual on DVE
                        nc.tensor.matmul(o_ps[:, :], mt8[:, :, csl],
                                         qs2[:, :, :], start=True, stop=True,
                                         perf_mode=DR)
                        oc = sp.tile([128, 512], F16, name=f"oc{s}_{b}_{c}",
                                     tag="oc", bufs=6)
                        nc.vector.scalar_tensor_tensor(
                            out=oc[:, :], in0=o_ps[:, :], scalar=ODS,
                            in1=xb16[:, c, :], op0=ALU.mult, op1=ALU.add)
                    else:
                        # residual folded into PSUM via 2048*I fp16 matmul,
                        # descale via ACT copy
                        nc.tensor.matmul(o_ps[:, :], mt8[:, :, csl],
                                         qs2[:, :, :], start=True, stop=False,
                                         perf_mode=DR)
                        nc.tensor.matmul(o_ps[:, :], idr16[:, :],
                                         xb16[:, c, :],
                                         start=False, stop=True)
                        oc = sp.tile([128, 512], F16, name=f"oc{s}_{b}_{c}",
                                     tag="oc", bufs=6)
                        nc.scalar.activation(oc[:, :], o_ps[:, :], AF.Copy,
                                             scale=ODS)
                    nc.sync.dma_start(
                        out=out_d[row0 + c * 128:row0 + (c + 1) * 128, bsl],
                        in_=oc[:, :])

            def phaseA_bank(s, b):
                pend.append((s, b) + _softmaxA(s, b))
                if len(pend) > 2:
                    _outputA(*pend.pop(0))

            def phaseA_flush():
                while pend:
                    _outputA(*pend.pop(0))

            # schedule: identical shape to the f32r baseline
            for g in range(NB):
                stage1_group(0, g)
            stage1_group(1, 0)
            stage1_group(1, 1)
            mid(0)
            for i in range(2, NB):
                if i >= 4:
                    stage1_group(1, i, pairs=(0,))
                    pend.append((0, i - 4) + _softmaxA(0, i - 4))
                    stage1_group(1, i, pairs=(1,))
                    if len(pend) > 2:
                        _outputA(*pend.pop(0))
                else:
                    stage1_group(1, i)
            mid(1)
            phaseA_bank(0, 4)
            for i in range(NB):
                phaseA_bank(1, i)
                if i in (0, 2, 4):
                    phaseA_bank(0, 5 + i // 2)
            phaseA_flush()
    nc.compile()
    return nc


def _host_prep(Wk, bk, Wq, bq, Wv, bv, We, be):
    import ml_dtypes
    f = np.float32
    F8 = ml_dtypes.float8_e4m3

    def chunk8(w):                  # (O, Cin) -> (128, Cin//128, O) fp8, x16
        wt = np.ascontiguousarray(w.T.astype(np.float64) * WS)
        wt = np.clip(wt, -240.0, 240.0)
        nch = wt.shape[0] // 128
        return np.ascontiguousarray(
            wt.reshape(nch, 128, w.shape[0]).transpose(1, 0, 2)).astype(F8)

    wk8 = chunk8(Wk)
    wq8 = chunk8(Wq)
    wv8 = chunk8(Wv)
    we8 = chunk8(We)
    bq2 = np.ascontiguousarray(
        bq.astype(f).reshape(2, 128).T) + np.float32(ESH)
    wb = (We.astype(np.float64) @ bv.astype(np.float64)
          + be.astype(np.float64))
    bsum = np.zeros((128, 128), f)
    for p in range(128):
        bsum[p, (p // 32) * 32:(p // 32) * 32 + 32] = 1.0
    ident = np.eye(128, dtype=f)
    idr16 = (np.eye(128) * 2048.0).astype(np.float16)
    ones8 = np.ones((128, 2, 16), dtype=F8)
    eshv = np.full((128, 1), ESH, dtype=f)
    return dict(wk8=wk8, wq8=wq8, wv8=wv8, we8=we8, bqv=bq2, eshv=eshv,
                bsum8=bsum.astype(F8), ident=ident, idr16=idr16,
                ones8=ones8), wb


def _make_in_maps(x, Wk, bk, Wq, bq, Wv, bv, We, be):
    import ml_dtypes
    F8 = ml_dtypes.float8_e4m3
    shared, wb = _host_prep(Wk, bk, Wq, bq, Wv, bv, We, be)
    xf = np.ascontiguousarray(x.astype(np.float64).reshape(N, C, L))
    x8 = np.clip(xf, -240.0, 240.0).astype(F8)
    x16 = (xf + wb[None, :, None]).astype(np.float16)
    in_maps = []
    for i in range(N_CORES):
        m = dict(shared)
        sl = slice(i * S_PER_CORE, (i + 1) * S_PER_CORE)
        m["xin8"] = np.ascontiguousarray(
            x8[sl].reshape(S_PER_CORE * C, L))
        m["xin16"] = np.ascontiguousarray(
            x16[sl].reshape(S_PER_CORE * C, L))
        in_maps.append(m)
    return in_maps


def kernel(x, Wk, bk, Wq, bq, Wv, bv, We, be):
    from concourse.bass_utils import run_bass_kernel_spmd

    assert x.shape == (N, C, Hdim, Wdim), x.shape
    if "nc" not in _CACHE:
        _CACHE["nc"] = _build_nc()
    nc = _CACHE["nc"]

    in_maps = _make_in_maps(x, Wk, bk, Wq, bq, Wv, bv, We, be)
    res = run_bass_kernel_spmd(nc, in_maps, core_ids=list(range(N_CORES)))
    out = np.concatenate(
        [np.asarray(res.results[i]["out"], dtype=np.float32)
         .reshape(S_PER_CORE, C, Hdim, Wdim)
         for i in range(N_CORES)], axis=0)
    return out.astype(np.float32)
